# revision 35
# baseline (speedup 1.0000x reference)
import os
import sys

for _p in ("/opt/trn_rl_repo", "/root/.axon_site/_ro/trn_rl_repo"):
    if os.path.isdir(_p) and _p not in sys.path:
        sys.path.insert(0, _p)

import numpy as np

N_CORES = 8
T_FULL = 16384
T = T_FULL // N_CORES      # 2048 tokens per core
D = 7168
E = 256
KT = D // 128              # 56 contraction tiles
TT = T // 128              # 16 token tiles per core
CW = 256                   # tokens per chunk
CH = CW // 128             # token tiles per chunk (2)
NCHUNK = 7                 # full-size chunks per core (rest: 2 x 128-token)
CWQ = 128                  # tail chunk width
WSPLIT = 4                 # weight DMA split (faster ramp)

N_GROUPS = 8
GROUP_SIZE = E // N_GROUPS  # 32
TOPK_GROUPS = 4
TOPK = 8
ROUTE_SCALE = 2.5
NEG_BIG = 1.0e30

_NC = None
TRACE = False
LAST_RESULT = None


def _ensure_axon_hooks():
    """The axon image lacks antenv.axon_hooks; bass_utils imports it when
    tracing is requested (e.g. BASS_TRACE=1). Install a working shim so a
    traced run profiles instead of crashing."""
    import importlib
    import types

    try:
        importlib.import_module("antenv.axon_hooks")
        return
    except ImportError:
        pass
    try:
        import antenv
    except ImportError:
        return
    mod = types.ModuleType("antenv.axon_hooks")
    state = {"hook": None}
    mod.set_axon_ntff_profile_hook = lambda h: state.__setitem__("hook", h)
    mod.get_axon_ntff_profile_hook = lambda: state["hook"]
    sys.modules["antenv.axon_hooks"] = mod
    antenv.axon_hooks = mod

    so_path = "/opt/axon/libaxon_pjrt.so"
    if os.path.exists(so_path):
        import contextlib
        import ctypes

        try:
            lib = ctypes.CDLL(so_path)
            if hasattr(lib, "axon_start_nrt_profile"):
                lib.axon_start_nrt_profile.argtypes = [
                    ctypes.POINTER(ctypes.c_int64), ctypes.c_size_t,
                ]
                lib.axon_start_nrt_profile.restype = ctypes.c_int64
                lib.axon_stop_nrt_profile.argtypes = [ctypes.c_char_p]
                lib.axon_stop_nrt_profile.restype = ctypes.c_int64

                @contextlib.contextmanager
                def _hook(output_dir, device_ids):
                    import jax

                    jax.devices()
                    if device_ids:
                        ids = (ctypes.c_int64 * len(device_ids))(*device_ids)
                        rc = lib.axon_start_nrt_profile(ids, len(device_ids))
                    else:
                        rc = lib.axon_start_nrt_profile(None, 0)
                    if rc != 0:
                        raise RuntimeError(f"axon_start_nrt_profile rc={rc}")
                    try:
                        yield
                    finally:
                        lib.axon_stop_nrt_profile(str(output_dir).encode())

                state["hook"] = _hook
        except OSError:
            pass


def _build_nc():
    import concourse.bass as bass
    import concourse.tile as tile
    from concourse import bacc, mybir

    nc = bacc.Bacc(None, target_bir_lowering=False)
    f32 = mybir.dt.float32
    f16 = mybir.dt.float16
    u32 = mybir.dt.uint32
    Alu = mybir.AluOpType

    # x packed per chunk: row (c*128+p), col (k*CW+t) = x[c*CW+t, k*128+p]
    xP = nc.dram_tensor("xP", [NCHUNK * 128, KT * CW], f16, kind="ExternalInput")
    xQ = nc.dram_tensor("xQ", [2 * 128, KT * CWQ], f16, kind="ExternalInput")
    # w packed: row p, col (k*E+e) = w[e, k*128+p]
    wP = nc.dram_tensor("wP", [128, KT * E], f16, kind="ExternalInput")
    bb = nc.dram_tensor("bb", [128, E], f32, kind="ExternalInput")
    # outputs partition-major: [p, tt*8+j] for token tt*128+p
    v8d = nc.dram_tensor("v8d", [128, TT * TOPK], f32, kind="ExternalOutput")
    i8d = nc.dram_tensor("i8d", [128, TT * TOPK], u32, kind="ExternalOutput")

    with tile.TileContext(nc) as tc:
        with (
            tc.tile_pool(name="w", bufs=1) as wpool,
            tc.tile_pool(name="c", bufs=1) as cpool,
            tc.tile_pool(name="x", bufs=2) as xpool,
            tc.tile_pool(name="s", bufs=6) as spool,
            tc.tile_pool(name="gm", bufs=6) as gmpool,
            tc.tile_pool(name="sm", bufs=18) as smpool,
            tc.tile_pool(name="o", bufs=1) as opool,
            tc.tile_pool(name="ps", bufs=2 * CH, space=bass.MemorySpace.PSUM) as pspool,
        ):
            # bias replicated across partitions
            bt = cpool.tile([128, E], f32)
            nc.scalar.dma_start(bt[:], bb[:, :])
            # constant -BIG tile for the penalty select
            nb8 = cpool.tile([128, N_GROUPS], f32)
            nc.vector.memset(nb8[:], -NEG_BIG)

            # resident gate weight (split DMA so first matmuls start sooner)
            wsb = wpool.tile([128, KT * E], f16)
            wstep = KT * E // WSPLIT
            for j in range(WSPLIT):
                nc.scalar.dma_start(
                    wsb[:, j * wstep:(j + 1) * wstep],
                    wP[:, j * wstep:(j + 1) * wstep],
                )

            # output staging (accumulated in SBUF, one DMA at the end)
            v8sb = opool.tile([128, TT * TOPK], f32)
            i8sb = opool.tile([128, TT * TOPK], u32)

            chunk_plan = [(c, CW, CH) for c in range(NCHUNK)] + [
                (0, CWQ, 1), (1, CWQ, 1)]
            for ci, (c, cw, ch) in enumerate(chunk_plan):
                xsplit = (4 if ci == 0 else 2) if cw == CW else 2
                xstep = KT * cw // xsplit
                src_t = xP if cw == CW else xQ
                xc = xpool.tile([128, KT * CW], f16)
                for j in range(xsplit):
                    nc.sync.dma_start(
                        xc[:, j * xstep:(j + 1) * xstep],
                        src_t[c * 128:(c + 1) * 128, j * xstep:(j + 1) * xstep],
                    )

                ps = []
                for t in range(ch):
                    p = pspool.tile([128, E], f32)
                    ps.append(p)

                # k-inner: 56 consecutive matmuls accumulate into ONE psum
                # bank (no psum-queue cycling between matmuls)
                for t in range(ch):
                    for k in range(KT):
                        nc.tensor.matmul(
                            ps[t][:],
                            xc[:, k * cw + t * 128:k * cw + (t + 1) * 128],
                            wsb[:, k * E:(k + 1) * E],
                            start=(k == 0), stop=(k == KT - 1),
                        )

                tt_base = ci * CH if cw == CW else NCHUNK * CH + c
                for t in range(ch):
                    tt = tt_base + t
                    # s = sigmoid(scores) + bias
                    s = spool.tile([128, E], f32)
                    nc.scalar.activation(
                        s[:], ps[t][:], mybir.ActivationFunctionType.Sigmoid
                    )
                    nc.gpsimd.tensor_add(s[:], s[:], bt[:])

                    # top-8 per group (only first two used)
                    gm = gmpool.tile([128, N_GROUPS, 8], f32)
                    for g in range(N_GROUPS):
                        nc.vector.max(
                            out=gm[:, g, :],
                            in_=s[:, g * GROUP_SIZE:(g + 1) * GROUP_SIZE],
                        )
                    # group score = top1 + top2; threshold = 4th largest
                    gs = smpool.tile([128, N_GROUPS], f32)
                    nc.vector.tensor_add(gs[:], gm[:, :, 0], gm[:, :, 1])
                    g8 = smpool.tile([128, 8], f32)
                    nc.vector.max(out=g8[:], in_=gs[:])
                    # additive penalty per group: (gs < t4) * -BIG
                    pen = smpool.tile([128, N_GROUPS], f32)
                    nc.vector.scalar_tensor_tensor(
                        pen[:], gs[:], g8[:, TOPK_GROUPS - 1:TOPK_GROUPS], nb8[:],
                        op0=Alu.is_lt, op1=Alu.mult,
                    )
                    # mask non-selected groups: one op via broadcast view
                    s3 = s[:].rearrange("p (g e) -> p g e", g=N_GROUPS)
                    nc.gpsimd.tensor_add(
                        s3, s3,
                        pen[:].unsqueeze(2).to_broadcast([128, N_GROUPS, GROUP_SIZE]),
                    )
                    # global top-8 values + indices
                    v8 = v8sb[:, tt * TOPK:(tt + 1) * TOPK]
                    nc.vector.max(out=v8, in_=s[:])
                    i8 = i8sb[:, tt * TOPK:(tt + 1) * TOPK]
                    nc.vector.max_index(out=i8, in_max=v8, in_values=s[:])

            nc.scalar.dma_start(v8d[:, :], v8sb[:])
            nc.scalar.dma_start(i8d[:, :], i8sb[:])

    nc.compile()
    return nc


def _get_nc():
    global _NC
    if _NC is None:
        _NC = _build_nc()
    return _NC


def _pack(xi16: np.ndarray, nchunk: int, cw: int) -> np.ndarray:
    # [tokens, D] -> [nchunk*128, KT*cw] with row (c*128+p), col (k*cw+t)
    return np.ascontiguousarray(
        xi16.reshape(nchunk, cw, KT, 128).transpose(0, 3, 2, 1)
    ).reshape(nchunk * 128, KT * cw)


def kernel(x: np.ndarray, weight: np.ndarray, bias: np.ndarray):
    global LAST_RESULT
    _ensure_axon_hooks()
    from concourse import bass_utils

    nc = _get_nc()
    bias = bias.astype(np.float32)
    x16 = x.astype(np.float16)
    wP = np.ascontiguousarray(
        weight.astype(np.float16).T.reshape(KT, 128, E).transpose(1, 0, 2)
    ).reshape(128, KT * E)
    bb = np.ascontiguousarray(
        np.broadcast_to(bias[None, :], (128, E)), dtype=np.float32
    )
    in_maps = []
    for i in range(N_CORES):
        xi = x16[i * T:(i + 1) * T]
        split = NCHUNK * CW
        in_maps.append({
            "xP": _pack(xi[:split], NCHUNK, CW),
            "xQ": _pack(xi[split:], 2, CWQ),
            "wP": wP,
            "bb": bb,
        })
    res = bass_utils.run_bass_kernel_spmd(
        nc, in_maps, core_ids=list(range(N_CORES)), trace=TRACE
    )
    LAST_RESULT = res
    # unpack outputs: [128, TT*8] -> [T, 8] with token tt*128+p
    v8 = np.concatenate(
        [r["v8d"].reshape(128, TT, TOPK).transpose(1, 0, 2).reshape(T, TOPK)
         for r in res.results], axis=0)
    i8 = np.concatenate(
        [r["i8d"].reshape(128, TT, TOPK).transpose(1, 0, 2).reshape(T, TOPK)
         for r in res.results], axis=0)
    idx = i8.astype(np.int32)
    sig8 = v8 - bias[idx]
    w8 = sig8 / sig8.sum(axis=-1, keepdims=True)
    w8 = (w8 * ROUTE_SCALE).astype(np.float32)
    return w8, idx


# revision 36
# speedup vs baseline: 1.0296x; 1.0296x over previous
import os
import sys

for _p in ("/opt/trn_rl_repo", "/root/.axon_site/_ro/trn_rl_repo"):
    if os.path.isdir(_p) and _p not in sys.path:
        sys.path.insert(0, _p)

import numpy as np

N_CORES = 8
T_FULL = 16384
T = T_FULL // N_CORES      # 2048 tokens per core
D = 7168
E = 256
KT = D // 128              # 56 contraction tiles
TT = T // 128              # 16 token tiles per core
CW = 256                   # tokens per chunk
CH = CW // 128             # token tiles per chunk (2)
NCHUNK = 7                 # full-size chunks per core (rest: 2 x 128-token)
CWQ = 128                  # tail chunk width
WSPLIT = 4                 # weight DMA split (faster ramp)

N_GROUPS = 8
GROUP_SIZE = E // N_GROUPS  # 32
TOPK_GROUPS = 4
TOPK = 8
ROUTE_SCALE = 2.5
NEG_BIG = 1.0e30

_NC = None
TRACE = False
LAST_RESULT = None


def _ensure_axon_hooks():
    """The axon image lacks antenv.axon_hooks; bass_utils imports it when
    tracing is requested (e.g. BASS_TRACE=1). Install a working shim so a
    traced run profiles instead of crashing."""
    import importlib
    import types

    try:
        importlib.import_module("antenv.axon_hooks")
        return
    except ImportError:
        pass
    try:
        import antenv
    except ImportError:
        return
    mod = types.ModuleType("antenv.axon_hooks")
    state = {"hook": None}
    mod.set_axon_ntff_profile_hook = lambda h: state.__setitem__("hook", h)
    mod.get_axon_ntff_profile_hook = lambda: state["hook"]
    sys.modules["antenv.axon_hooks"] = mod
    antenv.axon_hooks = mod

    so_path = "/opt/axon/libaxon_pjrt.so"
    if os.path.exists(so_path):
        import contextlib
        import ctypes

        try:
            lib = ctypes.CDLL(so_path)
            if hasattr(lib, "axon_start_nrt_profile"):
                lib.axon_start_nrt_profile.argtypes = [
                    ctypes.POINTER(ctypes.c_int64), ctypes.c_size_t,
                ]
                lib.axon_start_nrt_profile.restype = ctypes.c_int64
                lib.axon_stop_nrt_profile.argtypes = [ctypes.c_char_p]
                lib.axon_stop_nrt_profile.restype = ctypes.c_int64

                @contextlib.contextmanager
                def _hook(output_dir, device_ids):
                    import jax

                    jax.devices()
                    if device_ids:
                        ids = (ctypes.c_int64 * len(device_ids))(*device_ids)
                        rc = lib.axon_start_nrt_profile(ids, len(device_ids))
                    else:
                        rc = lib.axon_start_nrt_profile(None, 0)
                    if rc != 0:
                        raise RuntimeError(f"axon_start_nrt_profile rc={rc}")
                    try:
                        yield
                    finally:
                        lib.axon_stop_nrt_profile(str(output_dir).encode())

                state["hook"] = _hook
        except OSError:
            pass


def _build_nc():
    import concourse.bass as bass
    import concourse.tile as tile
    from concourse import bacc, mybir

    nc = bacc.Bacc(None, target_bir_lowering=False)
    f32 = mybir.dt.float32
    f16 = mybir.dt.float16
    u32 = mybir.dt.uint32
    Alu = mybir.AluOpType

    # x packed per chunk: row (c*128+p), col (k*CW+t) = x[c*CW+t, k*128+p]
    xP = nc.dram_tensor("xP", [NCHUNK * 128, KT * CW], f16, kind="ExternalInput")
    xQ = nc.dram_tensor("xQ", [2 * 128, KT * CWQ], f16, kind="ExternalInput")
    # w packed: row p, col (k*E+e) = w[e, k*128+p]
    wP = nc.dram_tensor("wP", [128, KT * E], f16, kind="ExternalInput")
    bb = nc.dram_tensor("bb", [128, E], f32, kind="ExternalInput")
    # outputs partition-major: [p, tt*8+j] for token tt*128+p
    v8d = nc.dram_tensor("v8d", [128, TT * TOPK], f32, kind="ExternalOutput")
    i8d = nc.dram_tensor("i8d", [128, TT * TOPK], u32, kind="ExternalOutput")

    with tile.TileContext(nc) as tc:
        with (
            tc.tile_pool(name="w", bufs=1) as wpool,
            tc.tile_pool(name="c", bufs=1) as cpool,
            tc.tile_pool(name="x", bufs=2) as xpool,
            tc.tile_pool(name="s", bufs=6) as spool,
            tc.tile_pool(name="gm", bufs=6) as gmpool,
            tc.tile_pool(name="sm", bufs=18) as smpool,
            tc.tile_pool(name="o", bufs=1) as opool,
            tc.tile_pool(name="ps", bufs=2 * CH, space=bass.MemorySpace.PSUM) as pspool,
        ):
            # bias replicated across partitions
            bt = cpool.tile([128, E], f32)
            nc.scalar.dma_start(bt[:], bb[:, :])
            # constant -BIG tile for the penalty select
            nb8 = cpool.tile([128, N_GROUPS], f32)
            nc.vector.memset(nb8[:], -NEG_BIG)

            # resident gate weight (split DMA so first matmuls start sooner)
            wsb = wpool.tile([128, KT * E], f16)
            wstep = KT * E // WSPLIT
            for j in range(WSPLIT):
                nc.scalar.dma_start(
                    wsb[:, j * wstep:(j + 1) * wstep],
                    wP[:, j * wstep:(j + 1) * wstep],
                )

            # output staging (accumulated in SBUF, one DMA at the end)
            v8sb = opool.tile([128, TT * TOPK], f32)
            i8sb = opool.tile([128, TT * TOPK], u32)

            chunk_plan = [(c, CW, CH) for c in range(NCHUNK)] + [
                (0, CWQ, 1), (1, CWQ, 1)]
            for ci, (c, cw, ch) in enumerate(chunk_plan):
                xsplit = 4 if cw == CW else 2
                xstep = KT * cw // xsplit
                src_t = xP if cw == CW else xQ
                xc = xpool.tile([128, KT * CW], f16)
                for j in range(xsplit):
                    nc.sync.dma_start(
                        xc[:, j * xstep:(j + 1) * xstep],
                        src_t[c * 128:(c + 1) * 128, j * xstep:(j + 1) * xstep],
                    )

                ps = []
                for t in range(ch):
                    p = pspool.tile([128, E], f32)
                    ps.append(p)

                # k-inner: 56 consecutive matmuls accumulate into ONE psum
                # bank (no psum-queue cycling between matmuls)
                for t in range(ch):
                    for k in range(KT):
                        nc.tensor.matmul(
                            ps[t][:],
                            xc[:, k * cw + t * 128:k * cw + (t + 1) * 128],
                            wsb[:, k * E:(k + 1) * E],
                            start=(k == 0), stop=(k == KT - 1),
                        )

                tt_base = ci * CH if cw == CW else NCHUNK * CH + c
                for t in range(ch):
                    tt = tt_base + t
                    # s = sigmoid(scores) + bias
                    s = spool.tile([128, E], f32)
                    nc.scalar.activation(
                        s[:], ps[t][:], mybir.ActivationFunctionType.Sigmoid
                    )
                    nc.gpsimd.tensor_add(s[:], s[:], bt[:])

                    # top-8 per group (only first two used)
                    gm = gmpool.tile([128, N_GROUPS, 8], f32)
                    for g in range(N_GROUPS):
                        nc.vector.max(
                            out=gm[:, g, :],
                            in_=s[:, g * GROUP_SIZE:(g + 1) * GROUP_SIZE],
                        )
                    # group score = top1 + top2; threshold = 4th largest
                    gs = smpool.tile([128, N_GROUPS], f32)
                    nc.vector.tensor_add(gs[:], gm[:, :, 0], gm[:, :, 1])
                    g8 = smpool.tile([128, 8], f32)
                    nc.vector.max(out=g8[:], in_=gs[:])
                    # additive penalty per group: (gs < t4) * -BIG
                    pen = smpool.tile([128, N_GROUPS], f32)
                    nc.vector.scalar_tensor_tensor(
                        pen[:], gs[:], g8[:, TOPK_GROUPS - 1:TOPK_GROUPS], nb8[:],
                        op0=Alu.is_lt, op1=Alu.mult,
                    )
                    # mask non-selected groups: one op via broadcast view
                    s3 = s[:].rearrange("p (g e) -> p g e", g=N_GROUPS)
                    nc.gpsimd.tensor_add(
                        s3, s3,
                        pen[:].unsqueeze(2).to_broadcast([128, N_GROUPS, GROUP_SIZE]),
                    )
                    # global top-8 values + indices
                    v8 = v8sb[:, tt * TOPK:(tt + 1) * TOPK]
                    nc.vector.max(out=v8, in_=s[:])
                    i8 = i8sb[:, tt * TOPK:(tt + 1) * TOPK]
                    nc.vector.max_index(out=i8, in_max=v8, in_values=s[:])

            nc.scalar.dma_start(v8d[:, :], v8sb[:])
            nc.scalar.dma_start(i8d[:, :], i8sb[:])

    nc.compile()
    return nc


def _get_nc():
    global _NC
    if _NC is None:
        _NC = _build_nc()
    return _NC


def _pack(xi16: np.ndarray, nchunk: int, cw: int) -> np.ndarray:
    # [tokens, D] -> [nchunk*128, KT*cw] with row (c*128+p), col (k*cw+t)
    return np.ascontiguousarray(
        xi16.reshape(nchunk, cw, KT, 128).transpose(0, 3, 2, 1)
    ).reshape(nchunk * 128, KT * cw)


def kernel(x: np.ndarray, weight: np.ndarray, bias: np.ndarray):
    global LAST_RESULT
    _ensure_axon_hooks()
    from concourse import bass_utils

    nc = _get_nc()
    bias = bias.astype(np.float32)
    x16 = x.astype(np.float16)
    wP = np.ascontiguousarray(
        weight.astype(np.float16).T.reshape(KT, 128, E).transpose(1, 0, 2)
    ).reshape(128, KT * E)
    bb = np.ascontiguousarray(
        np.broadcast_to(bias[None, :], (128, E)), dtype=np.float32
    )
    in_maps = []
    for i in range(N_CORES):
        xi = x16[i * T:(i + 1) * T]
        split = NCHUNK * CW
        in_maps.append({
            "xP": _pack(xi[:split], NCHUNK, CW),
            "xQ": _pack(xi[split:], 2, CWQ),
            "wP": wP,
            "bb": bb,
        })
    res = bass_utils.run_bass_kernel_spmd(
        nc, in_maps, core_ids=list(range(N_CORES)), trace=TRACE
    )
    LAST_RESULT = res
    # unpack outputs: [128, TT*8] -> [T, 8] with token tt*128+p
    v8 = np.concatenate(
        [r["v8d"].reshape(128, TT, TOPK).transpose(1, 0, 2).reshape(T, TOPK)
         for r in res.results], axis=0)
    i8 = np.concatenate(
        [r["i8d"].reshape(128, TT, TOPK).transpose(1, 0, 2).reshape(T, TOPK)
         for r in res.results], axis=0)
    idx = i8.astype(np.int32)
    sig8 = v8 - bias[idx]
    w8 = sig8 / sig8.sum(axis=-1, keepdims=True)
    w8 = (w8 * ROUTE_SCALE).astype(np.float32)
    return w8, idx


# revision 37
# speedup vs baseline: 1.0362x; 1.0064x over previous
import os
import sys

for _p in ("/opt/trn_rl_repo", "/root/.axon_site/_ro/trn_rl_repo"):
    if os.path.isdir(_p) and _p not in sys.path:
        sys.path.insert(0, _p)

import numpy as np

N_CORES = 8
T_FULL = 16384
T = T_FULL // N_CORES      # 2048 tokens per core
D = 7168
E = 256
KT = D // 128              # 56 contraction tiles
TT = T // 128              # 16 token tiles per core
CW = 256                   # tokens per chunk
CH = CW // 128             # token tiles per chunk (2)
NCHUNK = 7                 # full-size chunks per core (rest: 2 x 128-token)
CWQ = 128                  # tail chunk width
WSPLIT = 4                 # weight DMA split (faster ramp)

N_GROUPS = 8
GROUP_SIZE = E // N_GROUPS  # 32
TOPK_GROUPS = 4
TOPK = 8
ROUTE_SCALE = 2.5
NEG_BIG = 1.0e30

_NC = None
TRACE = False
LAST_RESULT = None


def _ensure_axon_hooks():
    """The axon image lacks antenv.axon_hooks; bass_utils imports it when
    tracing is requested (e.g. BASS_TRACE=1). Install a working shim so a
    traced run profiles instead of crashing."""
    import importlib
    import types

    try:
        importlib.import_module("antenv.axon_hooks")
        return
    except ImportError:
        pass
    try:
        import antenv
    except ImportError:
        return
    mod = types.ModuleType("antenv.axon_hooks")
    state = {"hook": None}
    mod.set_axon_ntff_profile_hook = lambda h: state.__setitem__("hook", h)
    mod.get_axon_ntff_profile_hook = lambda: state["hook"]
    sys.modules["antenv.axon_hooks"] = mod
    antenv.axon_hooks = mod

    so_path = "/opt/axon/libaxon_pjrt.so"
    if os.path.exists(so_path):
        import contextlib
        import ctypes

        try:
            lib = ctypes.CDLL(so_path)
            if hasattr(lib, "axon_start_nrt_profile"):
                lib.axon_start_nrt_profile.argtypes = [
                    ctypes.POINTER(ctypes.c_int64), ctypes.c_size_t,
                ]
                lib.axon_start_nrt_profile.restype = ctypes.c_int64
                lib.axon_stop_nrt_profile.argtypes = [ctypes.c_char_p]
                lib.axon_stop_nrt_profile.restype = ctypes.c_int64

                @contextlib.contextmanager
                def _hook(output_dir, device_ids):
                    import jax

                    jax.devices()
                    if device_ids:
                        ids = (ctypes.c_int64 * len(device_ids))(*device_ids)
                        rc = lib.axon_start_nrt_profile(ids, len(device_ids))
                    else:
                        rc = lib.axon_start_nrt_profile(None, 0)
                    if rc != 0:
                        raise RuntimeError(f"axon_start_nrt_profile rc={rc}")
                    try:
                        yield
                    finally:
                        lib.axon_stop_nrt_profile(str(output_dir).encode())

                state["hook"] = _hook
        except OSError:
            pass


def _build_nc():
    import concourse.bass as bass
    import concourse.tile as tile
    from concourse import bacc, mybir

    nc = bacc.Bacc(None, target_bir_lowering=False)
    f32 = mybir.dt.float32
    f16 = mybir.dt.float16
    u32 = mybir.dt.uint32
    Alu = mybir.AluOpType

    # x packed per chunk: row (c*128+p), col (k*CW+t) = x[c*CW+t, k*128+p]
    xP = nc.dram_tensor("xP", [NCHUNK * 128, KT * CW], f16, kind="ExternalInput")
    xQ = nc.dram_tensor("xQ", [2 * 128, KT * CWQ], f16, kind="ExternalInput")
    # w packed: row p, col (k*E+e) = w[e, k*128+p]
    wP = nc.dram_tensor("wP", [128, KT * E], f16, kind="ExternalInput")
    bb = nc.dram_tensor("bb", [128, E], f32, kind="ExternalInput")
    # outputs partition-major: [p, tt*8+j] for token tt*128+p
    v8d = nc.dram_tensor("v8d", [128, TT * TOPK], f32, kind="ExternalOutput")
    i8d = nc.dram_tensor("i8d", [128, TT * TOPK], u32, kind="ExternalOutput")

    with tile.TileContext(nc) as tc:
        with (
            tc.tile_pool(name="w", bufs=1) as wpool,
            tc.tile_pool(name="c", bufs=1) as cpool,
            tc.tile_pool(name="x", bufs=2) as xpool,
            tc.tile_pool(name="s", bufs=6) as spool,
            tc.tile_pool(name="gm", bufs=6) as gmpool,
            tc.tile_pool(name="sm", bufs=18) as smpool,
            tc.tile_pool(name="o", bufs=1) as opool,
            tc.tile_pool(name="ps", bufs=2 * CH, space=bass.MemorySpace.PSUM) as pspool,
        ):
            # bias replicated across partitions
            bt = cpool.tile([128, E], f32)
            nc.scalar.dma_start(bt[:], bb[:, :])
            # constant -BIG tile for the penalty select
            nb8 = cpool.tile([128, N_GROUPS], f32)
            nc.vector.memset(nb8[:], -NEG_BIG)

            # resident gate weight (split DMA so first matmuls start sooner)
            wsb = wpool.tile([128, KT * E], f16)
            wstep = KT * E // WSPLIT
            for j in range(WSPLIT):
                nc.scalar.dma_start(
                    wsb[:, j * wstep:(j + 1) * wstep],
                    wP[:, j * wstep:(j + 1) * wstep],
                )

            # output staging (accumulated in SBUF, one DMA at the end)
            v8sb = opool.tile([128, TT * TOPK], f32)
            i8sb = opool.tile([128, TT * TOPK], u32)

            chunk_plan = [(c, CW, CH) for c in range(NCHUNK)] + [
                (0, CWQ, 1), (1, CWQ, 1)]
            for ci, (c, cw, ch) in enumerate(chunk_plan):
                xsplit = 4 if cw == CW else 2
                xstep = KT * cw // xsplit
                src_t = xP if cw == CW else xQ
                xc = xpool.tile([128, KT * CW], f16)
                for j in range(xsplit):
                    nc.sync.dma_start(
                        xc[:, j * xstep:(j + 1) * xstep],
                        src_t[c * 128:(c + 1) * 128, j * xstep:(j + 1) * xstep],
                    )

                ps = []
                for t in range(ch):
                    p = pspool.tile([128, E], f32)
                    ps.append(p)

                # k-inner: 56 consecutive matmuls accumulate into ONE psum
                # bank (no psum-queue cycling between matmuls)
                for t in range(ch):
                    for k in range(KT):
                        nc.tensor.matmul(
                            ps[t][:],
                            xc[:, k * cw + t * 128:k * cw + (t + 1) * 128],
                            wsb[:, k * E:(k + 1) * E],
                            start=(k == 0), stop=(k == KT - 1),
                        )

                tt_base = ci * CH if cw == CW else NCHUNK * CH + c
                for t in range(ch):
                    tt = tt_base + t
                    # s = sigmoid(scores) + bias
                    s = spool.tile([128, E], f32)
                    nc.scalar.activation(
                        s[:], ps[t][:], mybir.ActivationFunctionType.Sigmoid
                    )
                    nc.gpsimd.tensor_add(s[:], s[:], bt[:])

                    # top-8 per group (only first two used)
                    gm = gmpool.tile([128, N_GROUPS, 8], f32)
                    for g in range(N_GROUPS):
                        nc.vector.max(
                            out=gm[:, g, :],
                            in_=s[:, g * GROUP_SIZE:(g + 1) * GROUP_SIZE],
                        )
                    # group score = top1 + top2; threshold = 4th largest
                    gs = smpool.tile([128, N_GROUPS], f32)
                    nc.vector.tensor_add(gs[:], gm[:, :, 0], gm[:, :, 1])
                    g8 = smpool.tile([128, 8], f32)
                    nc.vector.max(out=g8[:], in_=gs[:])
                    # additive penalty per group: (gs < t4) * -BIG
                    pen = smpool.tile([128, N_GROUPS], f32)
                    nc.vector.scalar_tensor_tensor(
                        pen[:], gs[:], g8[:, TOPK_GROUPS - 1:TOPK_GROUPS], nb8[:],
                        op0=Alu.is_lt, op1=Alu.mult,
                    )
                    # mask non-selected groups: one op via broadcast view
                    s3 = s[:].rearrange("p (g e) -> p g e", g=N_GROUPS)
                    nc.gpsimd.tensor_add(
                        s3, s3,
                        pen[:].unsqueeze(2).to_broadcast([128, N_GROUPS, GROUP_SIZE]),
                    )
                    # global top-8 values + indices
                    v8 = v8sb[:, tt * TOPK:(tt + 1) * TOPK]
                    nc.vector.max(out=v8, in_=s[:])
                    i8 = i8sb[:, tt * TOPK:(tt + 1) * TOPK]
                    nc.vector.max_index(out=i8, in_max=v8, in_values=s[:])

                lo, hi = tt_base * TOPK, (tt_base + ch) * TOPK
                nc.scalar.dma_start(v8d[:, lo:hi], v8sb[:, lo:hi])
                nc.scalar.dma_start(i8d[:, lo:hi], i8sb[:, lo:hi])

    nc.compile()
    return nc


def _get_nc():
    global _NC
    if _NC is None:
        _NC = _build_nc()
    return _NC


def _pack(xi16: np.ndarray, nchunk: int, cw: int) -> np.ndarray:
    # [tokens, D] -> [nchunk*128, KT*cw] with row (c*128+p), col (k*cw+t)
    return np.ascontiguousarray(
        xi16.reshape(nchunk, cw, KT, 128).transpose(0, 3, 2, 1)
    ).reshape(nchunk * 128, KT * cw)


def kernel(x: np.ndarray, weight: np.ndarray, bias: np.ndarray):
    global LAST_RESULT
    _ensure_axon_hooks()
    from concourse import bass_utils

    nc = _get_nc()
    bias = bias.astype(np.float32)
    x16 = x.astype(np.float16)
    wP = np.ascontiguousarray(
        weight.astype(np.float16).T.reshape(KT, 128, E).transpose(1, 0, 2)
    ).reshape(128, KT * E)
    bb = np.ascontiguousarray(
        np.broadcast_to(bias[None, :], (128, E)), dtype=np.float32
    )
    in_maps = []
    for i in range(N_CORES):
        xi = x16[i * T:(i + 1) * T]
        split = NCHUNK * CW
        in_maps.append({
            "xP": _pack(xi[:split], NCHUNK, CW),
            "xQ": _pack(xi[split:], 2, CWQ),
            "wP": wP,
            "bb": bb,
        })
    res = bass_utils.run_bass_kernel_spmd(
        nc, in_maps, core_ids=list(range(N_CORES)), trace=TRACE
    )
    LAST_RESULT = res
    # unpack outputs: [128, TT*8] -> [T, 8] with token tt*128+p
    v8 = np.concatenate(
        [r["v8d"].reshape(128, TT, TOPK).transpose(1, 0, 2).reshape(T, TOPK)
         for r in res.results], axis=0)
    i8 = np.concatenate(
        [r["i8d"].reshape(128, TT, TOPK).transpose(1, 0, 2).reshape(T, TOPK)
         for r in res.results], axis=0)
    idx = i8.astype(np.int32)
    sig8 = v8 - bias[idx]
    w8 = sig8 / sig8.sum(axis=-1, keepdims=True)
    w8 = (w8 * ROUTE_SCALE).astype(np.float32)
    return w8, idx
